# revision 6
# baseline (speedup 1.0000x reference)
"""Per-entity linear head: out[n, e] = sum_h x[n, e, h] * W[e, h] + b[e].

Full inputs: cell_states (4, 512, 64, 1024) f32, W (64, 1024), b (64,).
Data-parallel over the flattened batch*seq dim across 8 cores; W/b are
tiny and replicated (host-duplicated to 128 partitions so no on-chip
broadcast is ever needed).

Per core: x_core viewed as [16384, 1024] rows.  SBUF tile tt holds row
128*tt + p on partition p, i.e. partition p = (n-sub, e) with
n = 2*tt + p//64 and e = p % 64.  One fused DVE scalar_tensor_tensor per
tile computes y[:, tt] = sum_h(x * w) in a single pass over the data
(the elementwise product is discarded into a stride-0 dummy); the bias
is added once at the end on the tiny [128, 128] result tile, which is
stored contiguously and untangled on the host with a free numpy
transpose.

Note: the fused DVE TENSOR_TENSOR_REDUCE (InstISA) compiles but faults
at runtime on this terminal; InstTensorScalarPtr (scalar_tensor_tensor)
with accum_out is the native-BIR equivalent and runs fine.
"""

import numpy as np

import concourse.bass as bass
import concourse.mybir as mybir
from concourse import bacc, bass_utils
from concourse.tile import TileContext

B, S, E, H = 4, 512, 64, 1024
N_CORES = 8
N = B * S                # 2048 flattened batch*seq rows
NPC = N // N_CORES       # 256 n-rows per core
R = NPC * E              # 16384 (n, e) rows of length H per core
P = 128                  # SBUF partitions
T = R // P               # 128 reduce tiles per core
G = 4                    # reduce tiles per DMA (2 MiB per DMA)
X_BUFS = 8


def build() -> bass.Bass:
    # Bacc (not raw Bass): its compile() pass splits multi-sem waits into
    # EventSemaphore instructions (walrus here allows 1 wait/instruction)
    # and codegens InstISA subclasses like TENSOR_TENSOR_REDUCE.
    nc = bacc.Bacc("TRN2", target_bir_lowering=False)
    x = nc.dram_tensor("x", [R, H], mybir.dt.float32, kind="ExternalInput")
    w = nc.dram_tensor("w", [P, H], mybir.dt.float32, kind="ExternalInput")
    bvec = nc.dram_tensor("bvec", [P, 1], mybir.dt.float32, kind="ExternalInput")
    y = nc.dram_tensor("y", [P, T], mybir.dt.float32, kind="ExternalOutput")

    # [T//G, P, G, H]: group g, partition p covers x row (g*G + t)*P + p
    xg = x.rearrange("(g t p) h -> g p t h", t=G, p=P)

    with TileContext(nc) as tc:
        with (
            tc.tile_pool(name="xpool", bufs=X_BUFS) as xpool,
            tc.tile_pool(name="consts", bufs=1) as consts,
            tc.tile_pool(name="scratch", bufs=4) as scratch,
        ):
            w_sb = consts.tile([P, H], mybir.dt.float32)
            nc.sync.dma_start(out=w_sb[:], in_=w[:])
            b_sb = consts.tile([P, 1], mybir.dt.float32)
            nc.sync.dma_start(out=b_sb[:], in_=bvec[:])
            y_sb = consts.tile([P, T], mybir.dt.float32)

            for g in range(T // G):
                xt = xpool.tile([P, G, H], mybir.dt.float32)
                nc.sync.dma_start(out=xt[:], in_=xg[g])
                for i in range(G):
                    tt = g * G + i
                    dummy = scratch.tile([P, 1], mybir.dt.float32)
                    nc.vector.scalar_tensor_tensor(
                        out=dummy.broadcast_to((P, H)),
                        in0=xt[:, i],
                        scalar=1.0,
                        in1=w_sb[:],
                        op0=mybir.AluOpType.mult,
                        op1=mybir.AluOpType.mult,
                        accum_out=y_sb[:, tt : tt + 1],
                    )
            # y += b (per-partition scalar), then store the whole result
            nc.vector.tensor_scalar_add(y_sb[:], y_sb[:], b_sb[:, 0:1])
            nc.sync.dma_start(out=y[:], in_=y_sb[:])
    nc.compile()
    return nc


def _prepare_in_maps(cell_states, W, b):
    x_all = np.ascontiguousarray(cell_states, dtype=np.float32).reshape(N, E, H)
    w2 = np.concatenate([W, W], axis=0).astype(np.float32, copy=False)
    b2 = np.concatenate([b, b]).astype(np.float32, copy=False).reshape(P, 1)
    in_maps = []
    for c in range(N_CORES):
        xc = x_all[c * NPC : (c + 1) * NPC].reshape(R, H)
        in_maps.append({"x": xc, "w": w2, "bvec": b2})
    return in_maps


def _unshard(per_core_y):
    outs = []
    for y_raw in per_core_y:
        # y_raw[p, tt] = out[2*tt + p//64, p%64]
        outs.append(np.asarray(y_raw).reshape(2, E, T).transpose(2, 0, 1).reshape(NPC, E))
    return np.concatenate(outs, axis=0).reshape(B, S, E).astype(np.float32, copy=False)


def kernel_with_results(trace=False, **inputs):
    nc = build()
    in_maps = _prepare_in_maps(inputs["cell_states"], inputs["W"], inputs["b"])
    res = bass_utils.run_bass_kernel_spmd(
        nc, in_maps, core_ids=list(range(N_CORES)), trace=trace
    )
    out = _unshard([r["y"] for r in res.results])
    return out, res


def kernel(**inputs) -> np.ndarray:
    out, _ = kernel_with_results(trace=False, **inputs)
    return out
